# revision 2
# baseline (speedup 1.0000x reference)
"""Trainium2 Bass kernel v2: causal MHA (B=4, S=2048, D=1024, H=16).

Sharding (8 cores): core c -> batch b = c//2, head-group g = c%2 (8 heads).
Host sums the two head-group partials per batch and adds bo + bv @ Wo.

v2 changes vs baseline:
  - projections (C units) interleaved into the attention stream so the PE
    fills softmax-exp wait gaps instead of idling
  - scores issued alternating head0/head1 so the two 64-row PE tiles
    ((0,0)/(64,0)) run concurrently
  - exp batched [128, 4, 512] (one ScalarE call per 4 key-blocks)
  - denominator via a leading ones-column in v (acc row 0), reciprocal +
    gpsimd partition_broadcast replaces recip-DMA + fp32 broadcast matmul
  - O-projection contracts K=128 (head pairs packed on partitions; odd-head
    attn shifted up via one SBUF DMA per query mega-tile)
"""

import numpy as np
import ml_dtypes

import concourse.bass as bass
import concourse.mybir as mybir
import concourse.tile as tile
from concourse import bacc
from concourse.bass_utils import run_bass_kernel_spmd

B, S, D, H = 4, 2048, 1024, 16
DH = D // H            # 64
HPC = 8                # heads per core
HID = HPC * DH         # 512 hidden dims per core
QT = 512               # query mega-tile
NI = S // QT           # 4 query mega-tiles
NKB = S // 128         # 16 key blocks
F32 = mybir.dt.float32

DT = mybir.dt.bfloat16
NPDT = ml_dtypes.bfloat16

_CACHE = {}


def _build_nc(loop_n=None, phases="CDO"):
    nc = bacc.Bacc("TRN2", target_bir_lowering=False, debug=False)

    xt_d = nc.dram_tensor("xt", [D, S], DT, kind="ExternalInput")   # host-transposed
    wq_d = nc.dram_tensor("wq", [D, HID], DT, kind="ExternalInput")
    wk_d = nc.dram_tensor("wk", [D, HID], DT, kind="ExternalInput")
    wv_d = nc.dram_tensor("wv", [D, HID], DT, kind="ExternalInput")
    wo_d = nc.dram_tensor("wo", [HID, D], DT, kind="ExternalInput")
    bq_d = nc.dram_tensor("bq", [HID], F32, kind="ExternalInput")
    bk_d = nc.dram_tensor("bk", [HID], F32, kind="ExternalInput")
    out_d = nc.dram_tensor("out", [S, D], F32, kind="ExternalOutput")

    with tile.TileContext(nc) as tc:
        with tc.tile_pool(name="persist", bufs=1) as persist:
            xT = persist.tile([128, 8, S], DT)          # xT[p, kt, t] = x[t, kt*128+p]
            qT = persist.tile([128, 4, S], DT)          # [dh-in-pair, pair, token]
            kT = persist.tile([128, 4, S], DT)
            v_sb = persist.tile([128, NKB, HPC, DH + 1], DT)  # + ones column
            wq_sb = persist.tile([128, 8, HID], DT)
            wk_sb = persist.tile([128, 8, HID], DT)
            wv_sb = persist.tile([128, 8, HID], DT)
            wo_sb = persist.tile([128, 4, D], DT)       # [(h dh), pair, dcol]
            bq_sb = persist.tile([128, 4], F32)
            bk_sb = persist.tile([128, 4], F32)
            ones_sb = persist.tile([65, 64], DT)

            nc.sync.dma_start(out=wq_sb, in_=wq_d.rearrange("(kt p) n -> p kt n", p=128))
            nc.sync.dma_start(out=wk_sb, in_=wk_d.rearrange("(kt p) n -> p kt n", p=128))
            nc.sync.dma_start(out=wv_sb, in_=wv_d.rearrange("(kt p) n -> p kt n", p=128))
            nc.sync.dma_start(
                out=wo_sb,
                in_=wo_d.rearrange("(pair h dh) n -> (h dh) pair n", pair=4, h=2, dh=64),
            )
            nc.sync.dma_start(out=bq_sb, in_=bq_d.rearrange("(h p) -> p h", p=128))
            nc.sync.dma_start(out=bk_sb, in_=bk_d.rearrange("(h p) -> p h", p=128))
            nc.vector.memset(v_sb[:, :, :, DH : DH + 1], 1.0)
            nc.vector.memset(ones_sb, 1.0)

            def load_xt():
                xtv = xt_d.rearrange("(kt p) t -> p kt t", p=128)
                for kt in range(8):
                    nc.sync.dma_start(out=xT[:, kt, :], in_=xtv[:, kt, :])

            def body():
                load_xt()
                with (
                    tc.tile_pool(name="sps", bufs=2, space="PSUM") as sps_pool,
                    tc.tile_pool(name="acc", bufs=1, space="PSUM") as acc_pool,
                    tc.tile_pool(name="prj", bufs=2, space="PSUM") as prj_pool,
                    tc.tile_pool(name="esc", bufs=3) as esc_pool,
                    tc.tile_pool(name="nrm", bufs=2) as nrm_pool,
                    tc.tile_pool(name="att", bufs=2) as att_pool,
                    tc.tile_pool(name="osb", bufs=2) as osb_pool,
                ):
                    # ---- projection work units (interleaved into attention) ----
                    def unit_qk(w_sb, b_sb, dst, p, ch):
                        ps = prj_pool.tile([128, 512], F32, tag="prj")
                        for kt in range(8):
                            nc.tensor.matmul(
                                ps,
                                lhsT=w_sb[:, kt, p * 128 : (p + 1) * 128],
                                rhs=xT[:, kt, ch * 512 : (ch + 1) * 512],
                                start=(kt == 0),
                                stop=(kt == 7),
                            )
                        nc.vector.tensor_scalar_add(
                            out=dst[:, p, ch * 512 : (ch + 1) * 512],
                            in0=ps,
                            scalar1=b_sb[:, p : p + 1],
                        )

                    def unit_v(tt):
                        ps = prj_pool.tile([128, 512], F32, tag="prj")
                        for kt in range(8):
                            nc.tensor.matmul(
                                ps,
                                lhsT=xT[:, kt, tt * 128 : (tt + 1) * 128],
                                rhs=wv_sb[:, kt, :],
                                start=(kt == 0),
                                stop=(kt == 7),
                            )
                        nc.vector.tensor_copy(
                            out=v_sb[:, tt, :, 0:DH],
                            in_=ps.rearrange("p (h d) -> p h d", h=HPC),
                        )

                    def c_units():
                        # round r: q/k projections for token-chunk r (per
                        # pair) + v projections for key-blocks 4r..4r+3.
                        # unit index of Cq(pair, r) = 12r + 2*pair + 1;
                        # Cv(4r + m) = 12r + 8 + m.
                        for r in range(4):
                            for p in range(4):
                                yield lambda p=p, r=r: unit_qk(wk_sb, bk_sb, kT, p, r)
                                yield lambda p=p, r=r: unit_qk(wq_sb, bq_sb, qT, p, r)
                            for tt in range(4 * r, 4 * r + 4):
                                yield lambda tt=tt: unit_v(tt)

                    units = c_units()
                    issued = 0

                    def pop_unit(n=1):
                        nonlocal issued
                        for _ in range(n):
                            u = next(units, None)
                            if u is None:
                                return
                            issued += 1
                            u()

                    def ensure(n):
                        if issued < n:
                            pop_unit(n - issued)

                    if "D" not in phases:
                        pop_unit(48)
                        nc.gpsimd.dma_start(out=out_d[0:128, :], in_=xT[:, 0, 0:D])
                        if "C" in phases:
                            nc.gpsimd.dma_start(out=out_d[128:256, :], in_=qT[:, 0, 0:D])
                            nc.gpsimd.dma_start(out=out_d[256:384, :], in_=kT[:, 0, 0:D])
                            nc.gpsimd.dma_start(out=out_d[384:512, 0:520],
                                                in_=v_sb[:, 0, :, :])
                        return

                    # ---- attention ----
                    for i in range(NI):
                        attnT = att_pool.tile([128, 4, QT], DT, tag="attnT")
                        odd_st = att_pool.tile([64, 4, QT], DT, tag="odd")
                        qs = slice(i * QT, (i + 1) * QT)
                        nj = (i + 1) * (QT // 128)
                        band = nj - QT // 128
                        for pair in range(4):
                            # q/k projections for (pair, chunk i) must precede
                            # this pair's scores
                            ensure(12 * i + 2 * pair + 2)
                            accs = [acc_pool.tile([65, QT], F32, tag=f"acc{h2}",
                                                  name=f"acc{h2}")
                                    for h2 in range(2)]

                            def issue_av(j, esc):
                                for h2 in range(2):
                                    head = 2 * pair + h2
                                    nc.tensor.matmul(
                                        accs[h2],
                                        lhsT=v_sb[:, j, head, :],
                                        rhs=esc[:, h2, :],
                                        start=(j == 0),
                                        stop=(j == nj - 1),
                                    )

                            pending = None
                            for j in range(nj):
                                if j >= band:
                                    # v for diag key-block j (round i)
                                    ensure(12 * i + 8 + (j - band) + 1)
                                # scores: one key-block per head; heads
                                # alternate so PE row-tiles (0,0)/(64,0)
                                # overlap
                                sps = sps_pool.tile([128, 2, QT], F32, tag="sps")
                                esc = esc_pool.tile([128, 2, QT], DT, tag="esc")
                                for h2 in range(2):
                                    hp = slice(h2 * 64, h2 * 64 + 64)
                                    nc.tensor.matmul(
                                        sps[:, h2, :],
                                        lhsT=kT[hp, pair, j * 128 : (j + 1) * 128],
                                        rhs=qT[hp, pair, qs],
                                        start=True,
                                        stop=True,
                                    )
                                nc.scalar.activation(
                                    out=esc, in_=sps,
                                    func=mybir.ActivationFunctionType.Exp,
                                    scale=0.125,
                                )
                                if j >= band:
                                    for h2 in range(2):
                                        nc.gpsimd.affine_select(
                                            out=esc[:, h2 : h2 + 1, :],
                                            in_=esc[:, h2 : h2 + 1, :],
                                            compare_op=mybir.AluOpType.is_ge,
                                            fill=0.0,
                                            base=-128 * (j - band),
                                            pattern=[[-128, 1], [1, QT]],
                                            channel_multiplier=-1,
                                        )
                                # AV trails one group so its exp/mask deps are
                                # already met when the PE reaches it
                                if pending is not None:
                                    issue_av(*pending)
                                if j % 2 == 0:
                                    pop_unit(1)
                                pending = (j, esc)
                            issue_av(*pending)
                            # normalize: acc row 64 is the softmax denominator.
                            # recip stays in-lane (partition 64); a K=1 bf16
                            # matmul with a ones row broadcasts it to
                            # partitions 0-63, reusing the just-freed acc bank.
                            for h2 in range(2):
                                accsb = nrm_pool.tile([65, QT], DT, tag=f"accsb{h2}")
                                nc.vector.tensor_copy(out=accsb, in_=accs[h2])
                                with nc.allow_low_precision(
                                    reason="softmax scale; bf16 recip adds "
                                    "~0.4% vs 2e-2 budget"
                                ):
                                    nc.vector.reciprocal(
                                        out=accsb[64:65, :], in_=accsb[64:65, :]
                                    )
                                bc = acc_pool.tile([65, QT], F32, tag=f"acc{h2}",
                                                   name=f"bc{h2}")
                                nc.tensor.matmul(
                                    bc[0:64, :],
                                    lhsT=ones_sb[64:65, :],
                                    rhs=accsb[64:65, :],
                                    start=True,
                                    stop=True,
                                )
                                dst = (attnT[0:64, pair, :] if h2 == 0
                                       else odd_st[:, pair, :])
                                nc.vector.tensor_mul(dst, accsb[0:64, :],
                                                     bc[0:64, :])
                        nc.sync.dma_start(out=attnT[64:128, :, :], in_=odd_st)
                        if "O" not in phases:
                            nc.gpsimd.dma_start(out=out_d[i * QT : i * QT + 64, 0:QT],
                                                in_=attnT[0:64, 0, :])
                            continue
                        # output projection: K=128 (head pairs packed)
                        for qc in range(QT // 128):
                            osb = osb_pool.tile([128, D], F32, tag="osb")
                            for nch in range(2):
                                ops = prj_pool.tile([128, 512], F32, tag="prj")
                                for pair in range(4):
                                    nc.tensor.matmul(
                                        ops,
                                        lhsT=attnT[:, pair, qc * 128 : (qc + 1) * 128],
                                        rhs=wo_sb[:, pair, nch * 512 : (nch + 1) * 512],
                                        start=(pair == 0),
                                        stop=(pair == 3),
                                    )
                                nc.vector.tensor_copy(
                                    out=osb[:, nch * 512 : (nch + 1) * 512], in_=ops
                                )
                            r0 = i * QT + qc * 128
                            nc.sync.dma_start(out=out_d[r0 : r0 + 128, :], in_=osb)
                    pop_unit(48)  # drain any stragglers

            if loop_n is None:
                body()
            else:
                with tc.For_i(0, loop_n, 1):
                    body()

    nc.compile()
    return nc


def get_nc(loop_n=None, phases="CDO"):
    key = ("nc", loop_n, phases)
    if key not in _CACHE:
        _CACHE[key] = _build_nc(loop_n, phases)
    return _CACHE[key]


def make_inputs(x, Wq, bq, Wk, bk, Wv, bv, Wo, bo):
    """Build the 8 per-core input maps (host-side sharding + x transpose)."""
    x = np.asarray(x, dtype=np.float32)
    wq_g = [np.ascontiguousarray(np.asarray(Wq)[:, g * HID : (g + 1) * HID]).astype(NPDT) for g in range(2)]
    wk_g = [np.ascontiguousarray(np.asarray(Wk)[:, g * HID : (g + 1) * HID]).astype(NPDT) for g in range(2)]
    wv_g = [np.ascontiguousarray(np.asarray(Wv)[:, g * HID : (g + 1) * HID]).astype(NPDT) for g in range(2)]
    wo_g = [np.ascontiguousarray(np.asarray(Wo)[g * HID : (g + 1) * HID, :]).astype(NPDT) for g in range(2)]
    bq_g = [np.ascontiguousarray(np.asarray(bq, dtype=np.float32)[g * HID : (g + 1) * HID]) for g in range(2)]
    bk_g = [np.ascontiguousarray(np.asarray(bk, dtype=np.float32)[g * HID : (g + 1) * HID]) for g in range(2)]
    xt_b = [np.ascontiguousarray(x[b].T).astype(NPDT) for b in range(B)]
    in_maps = []
    for c in range(8):
        b, g = c // 2, c % 2
        in_maps.append({
            "xt": xt_b[b], "wq": wq_g[g], "wk": wk_g[g], "wv": wv_g[g],
            "wo": wo_g[g], "bq": bq_g[g], "bk": bk_g[g],
        })
    return in_maps


def assemble(results, Wv_bias_term):
    out = np.empty((B, S, D), dtype=np.float32)
    for b in range(B):
        out[b] = results[2 * b]["out"] + results[2 * b + 1]["out"] + Wv_bias_term
    return out


def kernel(x, Wq, bq, Wk, bk, Wv, bv, Wo, bo):
    nc = get_nc()
    in_maps = make_inputs(x, Wq, bq, Wk, bk, Wv, bv, Wo, bo)
    res = run_bass_kernel_spmd(nc, in_maps, core_ids=list(range(8)))
    corr = (np.asarray(bv, dtype=np.float32) @ np.asarray(Wo, dtype=np.float32)
            + np.asarray(bo, dtype=np.float32))
    return assemble(res.results, corr)


# revision 3
# speedup vs baseline: 2.6533x; 2.6533x over previous
"""Trainium2 Bass kernel v2: causal MHA (B=4, S=2048, D=1024, H=16).

Sharding (8 cores): core c -> batch b = c//2, head-group g = c%2 (8 heads).
Host sums the two head-group partials per batch and adds bo + bv @ Wo.

v2 changes vs baseline:
  - projections (C units) interleaved into the attention stream so the PE
    fills softmax-exp wait gaps instead of idling
  - scores issued alternating head0/head1 so the two 64-row PE tiles
    ((0,0)/(64,0)) run concurrently
  - exp batched [128, 4, 512] (one ScalarE call per 4 key-blocks)
  - denominator via a leading ones-column in v (acc row 0), reciprocal +
    gpsimd partition_broadcast replaces recip-DMA + fp32 broadcast matmul
  - O-projection contracts K=128 (head pairs packed on partitions; odd-head
    attn shifted up via one SBUF DMA per query mega-tile)
"""

import numpy as np
import ml_dtypes

import concourse.bass as bass
import concourse.mybir as mybir
import concourse.tile as tile
from concourse import bacc
from concourse.bass_utils import run_bass_kernel_spmd

B, S, D, H = 4, 2048, 1024, 16
DH = D // H            # 64
HPC = 8                # heads per core
HID = HPC * DH         # 512 hidden dims per core
QT = 512               # query mega-tile
NI = S // QT           # 4 query mega-tiles
NKB = S // 128         # 16 key blocks
F32 = mybir.dt.float32

DT = mybir.dt.bfloat16
NPDT = ml_dtypes.bfloat16

_CACHE = {}


def _build_nc(loop_n=None, phases="CDO"):
    nc = bacc.Bacc("TRN2", target_bir_lowering=False, debug=False)

    xt_d = nc.dram_tensor("xt", [D, S], DT, kind="ExternalInput")   # host-transposed
    wq_d = nc.dram_tensor("wq", [D, HID], DT, kind="ExternalInput")
    wk_d = nc.dram_tensor("wk", [D, HID], DT, kind="ExternalInput")
    wv_d = nc.dram_tensor("wv", [D, HID], DT, kind="ExternalInput")
    wo_d = nc.dram_tensor("wo", [HID, D], DT, kind="ExternalInput")
    bq_d = nc.dram_tensor("bq", [HID], F32, kind="ExternalInput")
    bk_d = nc.dram_tensor("bk", [HID], F32, kind="ExternalInput")
    out_d = nc.dram_tensor("out", [S, D], F32, kind="ExternalOutput")

    with tile.TileContext(nc) as tc:
        with tc.tile_pool(name="persist", bufs=1) as persist:
            xT = persist.tile([128, 8, S], DT)          # xT[p, kt, t] = x[t, kt*128+p]
            qT = persist.tile([128, 4, S], DT)          # [dh-in-pair, pair, token]
            kT = persist.tile([128, 4, S], DT)
            v_sb = persist.tile([128, NKB, HPC, DH + 1], DT)  # + ones column
            wq_sb = persist.tile([128, 8, HID], DT)
            wk_sb = persist.tile([128, 8, HID], DT)
            wv_sb = persist.tile([128, 8, HID], DT)
            wo_sb = persist.tile([128, 4, D], DT)       # [(h dh), pair, dcol]
            bq_sb = persist.tile([128, 4], F32)
            bk_sb = persist.tile([128, 4], F32)
            ones_sb = persist.tile([65, 64], DT)

            nc.sync.dma_start(out=wq_sb, in_=wq_d.rearrange("(kt p) n -> p kt n", p=128))
            nc.sync.dma_start(out=wk_sb, in_=wk_d.rearrange("(kt p) n -> p kt n", p=128))
            nc.sync.dma_start(out=wv_sb, in_=wv_d.rearrange("(kt p) n -> p kt n", p=128))
            nc.sync.dma_start(
                out=wo_sb,
                in_=wo_d.rearrange("(pair h dh) n -> (h dh) pair n", pair=4, h=2, dh=64),
            )
            nc.sync.dma_start(out=bq_sb, in_=bq_d.rearrange("(h p) -> p h", p=128))
            nc.sync.dma_start(out=bk_sb, in_=bk_d.rearrange("(h p) -> p h", p=128))
            nc.vector.memset(v_sb[:, :, :, DH : DH + 1], 1.0)
            nc.vector.memset(ones_sb, 1.0)

            def load_xt():
                xtv = xt_d.rearrange("(kt p) t -> p kt t", p=128)
                for kt in range(8):
                    nc.sync.dma_start(out=xT[:, kt, :], in_=xtv[:, kt, :])

            def body():
                load_xt()
                with (
                    tc.tile_pool(name="sps", bufs=2, space="PSUM") as sps_pool,
                    tc.tile_pool(name="acc", bufs=1, space="PSUM") as acc_pool,
                    tc.tile_pool(name="prj", bufs=2, space="PSUM") as prj_pool,
                    tc.tile_pool(name="esc", bufs=4) as esc_pool,
                    tc.tile_pool(name="nrm", bufs=2) as nrm_pool,
                    tc.tile_pool(name="att", bufs=2) as att_pool,
                    tc.tile_pool(name="osb", bufs=2) as osb_pool,
                ):
                    # ---- projection work units (interleaved into attention) ----
                    def unit_qk(w_sb, b_sb, dst, p, ch):
                        ps = prj_pool.tile([128, 512], F32, tag="prj")
                        for kt in range(8):
                            nc.tensor.matmul(
                                ps,
                                lhsT=w_sb[:, kt, p * 128 : (p + 1) * 128],
                                rhs=xT[:, kt, ch * 512 : (ch + 1) * 512],
                                start=(kt == 0),
                                stop=(kt == 7),
                            )
                        nc.vector.tensor_scalar_add(
                            out=dst[:, p, ch * 512 : (ch + 1) * 512],
                            in0=ps,
                            scalar1=b_sb[:, p : p + 1],
                        )

                    def unit_v(tt):
                        ps = prj_pool.tile([128, 512], F32, tag="prj")
                        for kt in range(8):
                            nc.tensor.matmul(
                                ps,
                                lhsT=xT[:, kt, tt * 128 : (tt + 1) * 128],
                                rhs=wv_sb[:, kt, :],
                                start=(kt == 0),
                                stop=(kt == 7),
                            )
                        nc.vector.tensor_copy(
                            out=v_sb[:, tt, :, 0:DH],
                            in_=ps.rearrange("p (h d) -> p h d", h=HPC),
                        )

                    def c_units():
                        # round r: q/k projections for token-chunk r (per
                        # pair) + v projections for key-blocks 4r..4r+3.
                        # unit index of Cq(pair, r) = 12r + 2*pair + 1;
                        # Cv(4r + m) = 12r + 8 + m.
                        for r in range(4):
                            for p in range(4):
                                yield lambda p=p, r=r: unit_qk(wk_sb, bk_sb, kT, p, r)
                                yield lambda p=p, r=r: unit_qk(wq_sb, bq_sb, qT, p, r)
                            for tt in range(4 * r, 4 * r + 4):
                                yield lambda tt=tt: unit_v(tt)

                    units = c_units()
                    issued = 0

                    def pop_unit(n=1):
                        nonlocal issued
                        for _ in range(n):
                            u = next(units, None)
                            if u is None:
                                return
                            issued += 1
                            u()

                    def ensure(n):
                        if issued < n:
                            pop_unit(n - issued)

                    if "D" not in phases:
                        pop_unit(48)
                        nc.gpsimd.dma_start(out=out_d[0:128, :], in_=xT[:, 0, 0:D])
                        if "C" in phases:
                            nc.gpsimd.dma_start(out=out_d[128:256, :], in_=qT[:, 0, 0:D])
                            nc.gpsimd.dma_start(out=out_d[256:384, :], in_=kT[:, 0, 0:D])
                            nc.gpsimd.dma_start(out=out_d[384:512, 0:520],
                                                in_=v_sb[:, 0, :, :])
                        return

                    # ---- attention ----
                    for i in range(NI):
                        attnT = att_pool.tile([128, 4, QT], DT, tag="attnT")
                        odd_st = att_pool.tile([64, 4, QT], DT, tag="odd")
                        qs = slice(i * QT, (i + 1) * QT)
                        nj = (i + 1) * (QT // 128)
                        band = nj - QT // 128
                        for pair in range(4):
                            # q/k projections for (pair, chunk i) must precede
                            # this pair's scores
                            ensure(12 * i + 2 * pair + 2)
                            accs = [acc_pool.tile([65, QT], F32, tag=f"acc{h2}",
                                                  name=f"acc{h2}")
                                    for h2 in range(2)]

                            def issue_av(j, esc):
                                for h2 in range(2):
                                    head = 2 * pair + h2
                                    nc.tensor.matmul(
                                        accs[h2],
                                        lhsT=v_sb[:, j, head, :],
                                        rhs=esc[:, h2, :],
                                        start=(j == 0),
                                        stop=(j == nj - 1),
                                    )

                            prev = []
                            for j0 in range(0, nj, 2):
                                cur = []
                                for j in (j0, j0 + 1):
                                    if j >= band:
                                        # v for diag key-block j (round i)
                                        ensure(12 * i + 8 + (j - band) + 1)
                                    # scores: one key-block per head; heads
                                    # alternate so PE row-tiles (0,0)/(64,0)
                                    # overlap; two blocks batched so the PE
                                    # switches tile mode half as often
                                    sps = sps_pool.tile([128, 2, QT], F32, tag="sps")
                                    esc = esc_pool.tile([128, 2, QT], DT, tag="esc")
                                    for h2 in range(2):
                                        hp = slice(h2 * 64, h2 * 64 + 64)
                                        nc.tensor.matmul(
                                            sps[:, h2, :],
                                            lhsT=kT[hp, pair, j * 128 : (j + 1) * 128],
                                            rhs=qT[hp, pair, qs],
                                            start=True,
                                            stop=True,
                                        )
                                    nc.scalar.activation(
                                        out=esc, in_=sps,
                                        func=mybir.ActivationFunctionType.Exp,
                                        scale=0.125,
                                    )
                                    if j >= band:
                                        for h2 in range(2):
                                            nc.gpsimd.affine_select(
                                                out=esc[:, h2 : h2 + 1, :],
                                                in_=esc[:, h2 : h2 + 1, :],
                                                compare_op=mybir.AluOpType.is_ge,
                                                fill=0.0,
                                                base=-128 * (j - band),
                                                pattern=[[-128, 1], [1, QT]],
                                                channel_multiplier=-1,
                                            )
                                    cur.append((j, esc))
                                # AV trails one batch so its exp/mask deps are
                                # already met when the PE reaches it
                                for pj, pesc in prev:
                                    issue_av(pj, pesc)
                                pop_unit(1)
                                prev = cur
                            for pj, pesc in prev:
                                issue_av(pj, pesc)
                            # normalize: acc row 64 is the softmax denominator.
                            # recip stays in-lane (partition 64); a K=1 bf16
                            # matmul with a ones row broadcasts it to
                            # partitions 0-63, reusing the just-freed acc bank.
                            for h2 in range(2):
                                accsb = nrm_pool.tile([65, QT], DT, tag=f"accsb{h2}")
                                nc.vector.tensor_copy(out=accsb, in_=accs[h2])
                                with nc.allow_low_precision(
                                    reason="softmax scale; bf16 recip adds "
                                    "~0.4% vs 2e-2 budget"
                                ):
                                    nc.vector.reciprocal(
                                        out=accsb[64:65, :], in_=accsb[64:65, :]
                                    )
                                bc = acc_pool.tile([65, QT], F32, tag=f"acc{h2}",
                                                   name=f"bc{h2}")
                                nc.tensor.matmul(
                                    bc[0:64, :],
                                    lhsT=ones_sb[64:65, :],
                                    rhs=accsb[64:65, :],
                                    start=True,
                                    stop=True,
                                )
                                dst = (attnT[0:64, pair, :] if h2 == 0
                                       else odd_st[:, pair, :])
                                nc.vector.tensor_mul(dst, accsb[0:64, :],
                                                     bc[0:64, :])
                        # SWDGE queue: keeps this small shift off the SP queue
                        # so O-proj isn't stuck behind the big out DMAs
                        nc.gpsimd.dma_start(out=attnT[64:128, :, :], in_=odd_st)
                        if "O" not in phases:
                            nc.gpsimd.dma_start(out=out_d[i * QT : i * QT + 64, 0:QT],
                                                in_=attnT[0:64, 0, :])
                            continue
                        # output projection: K=128 (head pairs packed)
                        for qc in range(QT // 128):
                            osb = osb_pool.tile([128, D], F32, tag="osb")
                            for nch in range(2):
                                ops = prj_pool.tile([128, 512], F32, tag="prj")
                                for pair in range(4):
                                    nc.tensor.matmul(
                                        ops,
                                        lhsT=attnT[:, pair, qc * 128 : (qc + 1) * 128],
                                        rhs=wo_sb[:, pair, nch * 512 : (nch + 1) * 512],
                                        start=(pair == 0),
                                        stop=(pair == 3),
                                    )
                                nc.vector.tensor_copy(
                                    out=osb[:, nch * 512 : (nch + 1) * 512], in_=ops
                                )
                            r0 = i * QT + qc * 128
                            nc.sync.dma_start(out=out_d[r0 : r0 + 128, :], in_=osb)
                    pop_unit(48)  # drain any stragglers

            if loop_n is None:
                body()
            else:
                with tc.For_i(0, loop_n, 1):
                    body()

    nc.compile()
    return nc


def get_nc(loop_n=None, phases="CDO"):
    key = ("nc", loop_n, phases)
    if key not in _CACHE:
        _CACHE[key] = _build_nc(loop_n, phases)
    return _CACHE[key]


def make_inputs(x, Wq, bq, Wk, bk, Wv, bv, Wo, bo):
    """Build the 8 per-core input maps (host-side sharding + x transpose)."""
    x = np.asarray(x, dtype=np.float32)
    wq_g = [np.ascontiguousarray(np.asarray(Wq)[:, g * HID : (g + 1) * HID]).astype(NPDT) for g in range(2)]
    wk_g = [np.ascontiguousarray(np.asarray(Wk)[:, g * HID : (g + 1) * HID]).astype(NPDT) for g in range(2)]
    wv_g = [np.ascontiguousarray(np.asarray(Wv)[:, g * HID : (g + 1) * HID]).astype(NPDT) for g in range(2)]
    wo_g = [np.ascontiguousarray(np.asarray(Wo)[g * HID : (g + 1) * HID, :]).astype(NPDT) for g in range(2)]
    bq_g = [np.ascontiguousarray(np.asarray(bq, dtype=np.float32)[g * HID : (g + 1) * HID]) for g in range(2)]
    bk_g = [np.ascontiguousarray(np.asarray(bk, dtype=np.float32)[g * HID : (g + 1) * HID]) for g in range(2)]
    xt_b = [np.ascontiguousarray(x[b].T).astype(NPDT) for b in range(B)]
    in_maps = []
    for c in range(8):
        b, g = c // 2, c % 2
        in_maps.append({
            "xt": xt_b[b], "wq": wq_g[g], "wk": wk_g[g], "wv": wv_g[g],
            "wo": wo_g[g], "bq": bq_g[g], "bk": bk_g[g],
        })
    return in_maps


def assemble(results, Wv_bias_term):
    out = np.empty((B, S, D), dtype=np.float32)
    for b in range(B):
        out[b] = results[2 * b]["out"] + results[2 * b + 1]["out"] + Wv_bias_term
    return out


def kernel(x, Wq, bq, Wk, bk, Wv, bv, Wo, bo):
    nc = get_nc()
    in_maps = make_inputs(x, Wq, bq, Wk, bk, Wv, bv, Wo, bo)
    res = run_bass_kernel_spmd(nc, in_maps, core_ids=list(range(8)))
    corr = (np.asarray(bv, dtype=np.float32) @ np.asarray(Wo, dtype=np.float32)
            + np.asarray(bo, dtype=np.float32))
    return assemble(res.results, corr)
